# revision 17
# baseline (speedup 1.0000x reference)
"""Multi-head attention (B=4, N=2048, C=1024, H=16) on 8 TRN2 NeuronCores.

Sharding: core = 2*b + half handles batch b, heads half*8 .. half*8+7.
Each core computes QKV for its 8 heads, full attention for them, and a
partial projection (its 512 rows of W_proj). Host sums the two partials
per batch and adds the bias.

v3 schedule: the scalar engine's exp stream (256 x [128,1024]) is the
critical resource; everything else is arranged so neither it nor the
PE ever hits a head-of-line block:
  - x^T is staged in DRAM as 32 contiguous (query-block, c-chunk)
    blocks and DMA'd in dependency-chained groups, so K^T/Q^T for the
    first query block (and the exp stream) start after ~1 MB of
    traffic instead of after the full 4 MB.
  - softmax normalization is PE-free and off the critical path: the
    denominators ride the AV matmuls as a 65th stationary column; the
    sums row is evicted, spread across 128 partitions via a DRAM
    round-trip, reciprocal'd wide (~0.2us instead of 6.5us on one
    partition), broadcast back with a stride-0 DMA and multiplied in
    on DVE.  These DMAs ride the gpsimd (SWDGE) queue so the sync
    queue never blocks on them.
  - V (pairs 0-1 beyond the first tiles), V (pairs 2-3), Q/K for later
    pairs and the first 12 projection tiles are emitted as small fill
    units inside the attention blocks, sized to the PE slack there.
  - PSUM evictions go to the scalar engine only where it is idle
    (boot, projection tail), otherwise to DVE.

All matmul operands are fp16 (1 cycle/row on the PE), accumulation
fp32 in PSUM. Host pre-casts weights/x and pre-transposes x; output
partials return as fp16 and are summed on the host in fp32.
"""

import functools
from contextlib import ExitStack

import numpy as np

import concourse.bass as bass
import concourse.tile as tile
from concourse.tile import add_dep_helper
from concourse import bacc, mybir
from concourse.bass_utils import run_bass_kernel_spmd

F32 = mybir.dt.float32
F16 = mybir.dt.float16
AF = mybir.ActivationFunctionType

B, N, C = 4, 2048, 1024
H, D = 16, 64
P = 128
NCORES = 8
HPC = 8            # heads per core
PAIRS = HPC // 2   # 4
DCORE = HPC * D    # 512 attention columns per core
SCALE = float(H) ** -0.5  # 0.25 (faithful to reference: num_heads**-0.5)
EXP_BIAS = -5.0    # exp(scale*s + bias): cancels in softmax, keeps fp16 range
NB = N // 512      # 4 query blocks
NT = N // P        # 16 key tiles of 128
CT = C // P        # 8 contraction chunks
VW = D + 1         # V columns per head incl. the ones column (row sums)
MBLK = HPC * VW    # 520 v_sb columns per m-tile
HB = DCORE // 2    # 256 V columns per half (head pairs 0-1 / 2-3)

LAST_RESULT = None  # BassKernelResults of the most recent run (for test.py)


def _kernel_body(tc, out_d, xtb_d, wq_d, wk_d, wv_d, wp_d):
    nc = tc.nc
    with ExitStack() as ctx:
        const = ctx.enter_context(tc.tile_pool(name="const", bufs=1))
        ones_f = const.tile([P, P], F32)
        nc.vector.memset(ones_f, 1.0)
        ebias = const.tile([P, 1], F32)
        nc.vector.memset(ebias, EXP_BIAS)

        # attT: pair p occupies cols [p*N, (p+1)*N); partitions = 2 heads x 64
        attT_pool = ctx.enter_context(tc.tile_pool(name="attT", bufs=1))
        attT = attT_pool.tile([P, PAIRS * N], F16)
        xt_pool = ctx.enter_context(tc.tile_pool(name="xt", bufs=1))
        xt = xt_pool.tile([P, CT * N], F16)
        v_pool = ctx.enter_context(tc.tile_pool(name="v", bufs=1))
        v_sb = v_pool.tile([P, NT * MBLK], F16)
        wv_pool = ctx.enter_context(tc.tile_pool(name="wv", bufs=1))
        wv_sb = wv_pool.tile([P, CT * DCORE], F16)
        wp_pool = ctx.enter_context(tc.tile_pool(name="wp", bufs=1))
        wp_sb = wp_pool.tile([P, PAIRS * C], F16)

        qt_pool = ctx.enter_context(tc.tile_pool(name="qt", bufs=3))
        kt_pool = ctx.enter_context(tc.tile_pool(name="kt", bufs=3))
        wqk_pool = ctx.enter_context(tc.tile_pool(name="wqk", bufs=4))
        pt_pool = ctx.enter_context(tc.tile_pool(name="pt", bufs=8))
        tmb_pool = ctx.enter_context(tc.tile_pool(name="tmb", bufs=3))
        sums_pool = ctx.enter_context(tc.tile_pool(name="sums", bufs=2))
        spread_pool = ctx.enter_context(tc.tile_pool(name="spread", bufs=2))
        rb_pool = ctx.enter_context(tc.tile_pool(name="rb", bufs=2))
        stage_pool = ctx.enter_context(tc.tile_pool(name="stage", bufs=3))
        dram_pool = ctx.enter_context(
            tc.tile_pool(name="dscr", bufs=3, space="DRAM"))

        ps_mm = ctx.enter_context(tc.tile_pool(name="ps_mm", bufs=2, space="PSUM"))

        qt_tiles = [None] * PAIRS
        kt_tiles = [None] * PAIRS
        wt_tiles = [None] * PAIRS

        def emit_wqk_dma(p):
            # host pre-packs the SBUF image: block p is a contiguous
            # [128, CT*P] slab, so this is a single dense transfer
            tiles = []
            for w_d in (wq_d, wk_d):
                wt = wqk_pool.tile([P, CT * P], F16, tag="w")
                nc.sync.dma_start(out=wt, in_=w_d[p * P:(p + 1) * P, :])
                tiles.append(wt)
            wt_tiles[p] = tiles

        def emit_wv_dma(half):
            # half-major packed image: one contiguous [128, CT*HB] transfer
            w = CT * HB
            return [nc.sync.dma_start(
                out=wv_sb[:, half * w:(half + 1) * w],
                in_=wv_d[:, half * w:(half + 1) * w])]

        # ---- boot DMAs, dependency-chained so the first query block's
        # x^T (1 MB) and wv(pairs 0-1) land before the rest of x^T ----
        emit_wqk_dma(0)
        xt_last = {}

        def emit_xt_group(nb, after=None, stage_pairs=False):
            # split each group across the HWDGE (sync) and SWDGE (gpsimd)
            # queue families to engage more DMA capacity during boot.
            # stage_pairs: chain chunks two at a time so they arrive
            # progressively (each dma_start fans over all 16 engines, so
            # serializing keeps full bandwidth) and the per-chunk K^T/Q^T
            # contraction matmuls can run during the transfer.
            last = None
            prev_stage = after
            for cc in range(CT):
                eng = nc.sync if cc % 2 == 0 else nc.gpsimd
                ins = eng.dma_start(
                    out=xt[:, cc * N + nb * 512: cc * N + nb * 512 + 512],
                    in_=xtb_d[(nb * CT + cc) * P:(nb * CT + cc + 1) * P, :])
                dep = prev_stage if (stage_pairs or cc < 2 or not stage_pairs
                                     ) else None
                if stage_pairs:
                    if prev_stage is not None:
                        add_dep_helper(ins.ins, prev_stage.ins, sync=True,
                                       reason="boot DMA pacing")
                    if cc % 2 == 1:
                        prev_stage = ins
                elif after is not None:
                    add_dep_helper(ins.ins, after.ins, sync=True,
                                   reason="boot DMA pacing")
                last = ins
            xt_last[nb] = last
            return last

        g_a = emit_xt_group(0, stage_pairs=True)
        wv01 = emit_wv_dma(0)
        for ins in wv01:
            add_dep_helper(ins.ins, g_a.ins, sync=True,
                           reason="boot DMA pacing")
        g_c = emit_xt_group(1, after=g_a)
        g_d = emit_xt_group(2, after=g_c)
        emit_xt_group(3, after=g_d)

        # ones columns of v_sb (fused softmax row sums)
        ones_cols = v_sb.rearrange("q (g k) -> q g k", k=VW)[:, :, D:VW]
        nc.vector.tensor_copy(
            ones_cols, ones_f.rearrange("q (g k) -> q g k", k=1))

        def emit_v_tile(half, m, evict_vector):
            base0 = half * CT * HB
            psv = ps_mm.tile([P, HB], F32, tag="mm")
            for cc in range(CT):
                nc.tensor.matmul(
                    psv,
                    xt[:, cc * N + m * P: cc * N + (m + 1) * P],
                    wv_sb[:, base0 + cc * HB: base0 + (cc + 1) * HB],
                    start=(cc == 0), stop=(cc == CT - 1))
            base = m * MBLK + 4 * half * VW
            dst = v_sb[:, base: base + 4 * VW].rearrange(
                "q (h k) -> q h k", k=VW)[:, :, 0:D]
            src = psv.rearrange("q (h k) -> q h k", k=D)
            if evict_vector:
                nc.vector.tensor_copy(dst, src)
            else:
                nc.scalar.copy(dst, src)

        def emit_qk_block(p, which, nb):
            # which: 0 = q, 1 = k
            wt = wt_tiles[p][which]
            dst = (qt_tiles if which == 0 else kt_tiles)[p]
            psq = ps_mm.tile([P, 512], F32, tag="mm")
            for cc in range(CT):
                nc.tensor.matmul(
                    psq,
                    wt[:, cc * P:(cc + 1) * P],
                    xt[:, cc * N + nb * 512: cc * N + nb * 512 + 512],
                    start=(cc == 0), stop=(cc == CT - 1))
            nc.vector.tensor_copy(dst[:, nb * 512:(nb + 1) * 512], psq)

        def emit_proj(i, evict_vector):
            for co in range(2):
                psp = ps_mm.tile([P, 512], F32, tag="mm")
                for dc in range(PAIRS):
                    nc.tensor.matmul(
                        psp,
                        attT[:, dc * N + i * P: dc * N + (i + 1) * P],
                        wp_sb[:, dc * C + co * 512: dc * C + co * 512 + 512],
                        start=(dc == 0), stop=(dc == PAIRS - 1))
                st = stage_pool.tile([P, 512], F16, tag="st")
                if evict_vector:
                    nc.vector.tensor_copy(st, psp)
                else:
                    nc.scalar.copy(st, psp)
                nc.sync.dma_start(
                    out=out_d[i * P:(i + 1) * P, co * 512: co * 512 + 512],
                    in_=st)

        # ---- boot PE work: K^T(p0,b0), Q^T(p0,b0), V(pairs 0-1, m 0-3) ----
        qt_tiles[0] = qt_pool.tile([P, N], F16, tag="qt", name="qt0")
        kt_tiles[0] = kt_pool.tile([P, N], F16, tag="kt", name="kt0")
        emit_qk_block(0, 1, 0)
        emit_qk_block(0, 0, 0)
        emit_v_tile(0, 0, evict_vector=True)
        emit_v_tile(0, 1, evict_vector=True)

        ps_s = ctx.enter_context(tc.tile_pool(name="ps_s", bufs=2, space="PSUM"))
        ps_av = ctx.enter_context(tc.tile_pool(name="ps_av", bufs=1, space="PSUM"))

        # ---- fill units: small chunks of PE work dropped into the slack
        # of the exp-bound attention blocks ----
        def qtf(p, nb):
            return lambda: emit_qk_block(p, 0, nb)

        def ktf(p, nb):
            return lambda: emit_qk_block(p, 1, nb)

        def vf01(m):
            return lambda: emit_v_tile(0, m, evict_vector=True)

        def vf23(m):
            return lambda: emit_v_tile(1, m, evict_vector=True)

        def projf(i):
            return lambda: emit_proj(i, evict_vector=True)

        def newpair(p):
            def f():
                qt_tiles[p] = qt_pool.tile([P, N], F16, tag="qt", name=f"qt{p}")
                kt_tiles[p] = kt_pool.tile([P, N], F16, tag="kt", name=f"kt{p}")
                emit_wqk_dma(p)
            return f

        def misc1():  # wv(pairs 2-3) + pair-1 weights
            def f():
                emit_wv_dma(1)
                newpair(1)()
            return f

        def misc2():  # pair-3 weights + W_proj
            def f():
                newpair(3)()
                for dc in range(PAIRS):
                    nc.sync.dma_start(out=wp_sb[:, dc * C:(dc + 1) * C],
                                      in_=wp_d[dc * P:(dc + 1) * P, :])
            return f

        FILLS = {
            (0, 0): [vf01(2), vf01(3), ktf(0, 1), vf01(4), vf01(5),
                     vf01(6), ktf(0, 2), vf01(7), vf01(8), ktf(0, 3),
                     vf01(9), vf01(10), qtf(0, 1), vf01(11), vf01(12),
                     vf01(13), vf01(14), vf01(15)],
            (0, 1): [misc1(), qtf(0, 2)],
            (0, 2): [qtf(0, 3), vf23(0), vf23(1), vf23(2)],
            (0, 3): [ktf(1, 0), ktf(1, 1), qtf(1, 0), vf23(3), vf23(4)],
            (1, 0): [ktf(1, 2), ktf(1, 3), qtf(1, 1), vf23(5), vf23(6)],
            (1, 1): [newpair(2), misc2(), qtf(1, 2), vf23(7), vf23(8)],
            (1, 2): [qtf(1, 3), ktf(2, 0), ktf(2, 1), vf23(9), vf23(10)],
            (1, 3): [ktf(2, 2), ktf(2, 3), qtf(2, 0), vf23(11), vf23(12)],
            (2, 0): [vf23(13), vf23(14), vf23(15), qtf(2, 1), ktf(3, 0)],
            (2, 1): [ktf(3, 1), ktf(3, 2), ktf(3, 3), qtf(3, 0), qtf(2, 2)],
            (3, 0): [qtf(2, 3), qtf(3, 1)],
            (2, 2): [qtf(3, 2), projf(0), projf(1)],
            (3, 1): [projf(2), projf(3), qtf(3, 3)],
            (3, 2): [projf(4), projf(5)],
            (2, 3): [projf(6), projf(7), projf(8)],
            (3, 3): [projf(9), projf(10), projf(11)],
        }
        # pair 2/3 interleave: pair-3 norms land early enough that the
        # projection spreads over the last five blocks instead of piling
        # up after the exp stream ends
        BLOCK_ORDER = ([(0, nb) for nb in range(NB)]
                       + [(1, nb) for nb in range(NB)]
                       + [(2, 0), (2, 1), (3, 0), (2, 2),
                          (3, 1), (3, 2), (2, 3), (3, 3)])
        # fills consumed per m-slot: (0,0) takes 2 per slot early (it is
        # DMA-paced anyway); other blocks 1 at each of 5 points.
        DENSE_FILL_CNT = {m: (2 if m < 8 else 1) for m in range(16)}
        FILL_CNT = {2: 1, 5: 1, 8: 1, 11: 1, 14: 1}

        def emit_c_block(p, nb):
            qt_t, kt_t = qt_tiles[p], kt_tiles[p]
            fills = list(FILLS.get((p, nb), []))
            fill_cnt = DENSE_FILL_CNT if (p, nb) == (0, 0) else FILL_CNT
            nsl = slice(nb * 512, nb * 512 + 512)
            osl = slice(p * N + nb * 512, p * N + nb * 512 + 512)
            ps_av_t = ps_av.tile([P, 1024], F32, tag="av")
            def emit_av(m, pt):
                first, last = (m == 0), (m == NT - 1)
                vbase = m * MBLK + 2 * p * VW
                nc.tensor.matmul(
                    ps_av_t[0:VW, 0:512],
                    v_sb[:, vbase: vbase + VW],
                    pt[:, 0:512],
                    start=first, stop=last, skip_group_check=True)
                nc.tensor.matmul(
                    ps_av_t[0:VW, 512:1024],
                    v_sb[:, vbase + VW: vbase + 2 * VW],
                    pt[:, 512:1024],
                    start=first, stop=last, skip_group_check=True)

            # AV trails scores/exp by 2 m-steps so the AV-start WAR on the
            # previous block's eviction never blocks the score stream
            av_q = []
            for m in range(NT):
                for _ in range(fill_cnt.get(m, 0)):
                    if fills:
                        fills.pop(0)()
                ps_s_t = ps_s.tile([P, 1024], F32, tag="s")
                nc.tensor.matmul(
                    ps_s_t[:, 0:512],
                    kt_t[0:64, m * P:(m + 1) * P], qt_t[0:64, nsl],
                    start=True, stop=True)
                nc.tensor.matmul(
                    ps_s_t[:, 512:1024],
                    kt_t[64:128, m * P:(m + 1) * P], qt_t[64:128, nsl],
                    start=True, stop=True)
                pt = pt_pool.tile([P, 1024], F16, tag="pt")
                nc.scalar.activation(pt, ps_s_t, AF.Exp,
                                     scale=SCALE, bias=ebias)
                av_q.append((m, pt))
                if len(av_q) > 2:
                    emit_av(*av_q.pop(0))
            while fills:
                fills.pop(0)()
            while av_q:
                emit_av(*av_q.pop(0))
            # evict each PSUM bank with ONE fp16 copy (rows 0-63 head data,
            # row 64 the softmax sums), so the next block's AV start
            # unblocks after ~0.7us per bank
            dmae = nc.sync if p == PAIRS - 1 else nc.gpsimd
            stA = sums_pool.tile([P, 512], F16, tag="stA")
            nc.vector.tensor_copy(stA[0:VW, :], ps_av_t[0:VW, 0:512])
            stB = sums_pool.tile([P, 512], F16, tag="stB")
            nc.vector.tensor_copy(stB[0:VW, :], ps_av_t[0:VW, 512:1024])
            dsum = dram_pool.tile([1024], F16, tag="dsum")
            dmae.dma_start(out=dsum.rearrange("(a b) -> a b", a=1)[:, 0:512],
                           in_=stA[64:65, :])
            dmae.dma_start(out=dsum.rearrange("(a b) -> a b", a=1)[:, 512:1024],
                           in_=stB[64:65, :])
            # normalization (PE-free): spread sums across 128 partitions via
            # DRAM, wide reciprocal, stride-0 broadcast back, multiply in.
            spread = spread_pool.tile([P, 8], F16, tag="spf")
            dmae.dma_start(out=spread,
                           in_=dsum.rearrange("(q f) -> q f", q=P))
            spreadr = spread_pool.tile([P, 8], F16, tag="sph")
            with nc.allow_low_precision(reason="softmax recip rounding"):
                nc.vector.reciprocal(spreadr, spread)
            drec = dram_pool.tile([1024], F16, tag="drec")
            dmae.dma_start(out=drec.rearrange("(q f) -> q f", q=P),
                           in_=spreadr)
            rb = rb_pool.tile([64, 1024], F16, tag="rb")
            dmae.dma_start(
                out=rb,
                in_=drec.rearrange("(a b) -> a b", a=1).broadcast_to([64, 1024]))
            nc.vector.tensor_mul(attT[0:64, osl], stA[0:64, :], rb[:, 0:512])
            tmb = tmb_pool.tile([64, 512], F16, tag="tmb")
            nc.vector.tensor_mul(tmb, stB[0:64, :], rb[:, 512:1024])
            nc.sync.dma_start(out=attT[64:128, osl], in_=tmb)

        for p, nb in BLOCK_ORDER:
            emit_c_block(p, nb)

        # ---- tail: projection of the last query block ----
        for i in range(4 * (NB - 1), 4 * NB):
            emit_proj(i, evict_vector=False)


@functools.lru_cache(maxsize=1)
def build_nc():
    nc = bacc.Bacc("TRN2", target_bir_lowering=False, debug=False)
    xtb_d = nc.dram_tensor("xt_blocks", [NB * CT * P, 512], F16,
                           kind="ExternalInput").ap()
    wq_d = nc.dram_tensor("wq", [PAIRS * P, CT * P], F16,
                          kind="ExternalInput").ap()
    wk_d = nc.dram_tensor("wk", [PAIRS * P, CT * P], F16,
                          kind="ExternalInput").ap()
    wv_d = nc.dram_tensor("wv", [P, 2 * CT * HB], F16,
                          kind="ExternalInput").ap()
    wp_d = nc.dram_tensor("wp", [DCORE, C], F16, kind="ExternalInput").ap()
    out_d = nc.dram_tensor("out_partial", [N, C], F16, kind="ExternalOutput").ap()
    with tile.TileContext(nc) as tc:
        _kernel_body(tc, out_d, xtb_d, wq_d, wk_d, wv_d, wp_d)
    nc.compile()
    return nc


def make_in_maps(x, W_qkv, W_proj):
    in_maps = []
    for core in range(NCORES):
        b, half = core // 2, core % 2
        h0 = half * HPC
        xt = x[b].T.astype(np.float16)              # [C, N]
        xtb = np.ascontiguousarray(
            xt.reshape(CT, P, NB, 512).transpose(2, 0, 1, 3)
        ).reshape(NB * CT * P, 512)

        def pack_qk(w):                              # [C, DCORE] -> SBUF image
            blocks = []
            for p_ in range(PAIRS):
                blk = w[:, p_ * P:(p_ + 1) * P]      # [C, 128]
                blocks.append(blk.reshape(CT, P, P).transpose(1, 0, 2)
                              .reshape(P, CT * P))
            return np.ascontiguousarray(np.concatenate(blocks, axis=0))

        wq_full = W_qkv[:, 0 * C + h0 * D: 0 * C + h0 * D + DCORE].astype(np.float16)
        wk_full = W_qkv[:, 1 * C + h0 * D: 1 * C + h0 * D + DCORE].astype(np.float16)
        wv_full = W_qkv[:, 2 * C + h0 * D: 2 * C + h0 * D + DCORE].astype(np.float16)
        wv_img = np.ascontiguousarray(
            wv_full.reshape(CT, P, 2, HB).transpose(1, 2, 0, 3)
        ).reshape(P, 2 * CT * HB)
        in_maps.append({
            "xt_blocks": xtb,
            "wq": pack_qk(wq_full),
            "wk": pack_qk(wk_full),
            "wv": wv_img,
            "wp": np.ascontiguousarray(
                W_proj[h0 * D: h0 * D + DCORE, :].astype(np.float16)),
        })
    return in_maps


def kernel(x, W_qkv, W_proj, b_proj, trace=False):
    x = np.asarray(x, dtype=np.float32)
    W_qkv = np.asarray(W_qkv, dtype=np.float32)
    W_proj = np.asarray(W_proj, dtype=np.float32)
    b_proj = np.asarray(b_proj, dtype=np.float32)

    nc = build_nc()
    in_maps = make_in_maps(x, W_qkv, W_proj)

    global LAST_RESULT
    res = run_bass_kernel_spmd(nc, in_maps, list(range(NCORES)), trace=trace)
    LAST_RESULT = res

    out = np.empty((B, N, C), dtype=np.float32)
    for b in range(B):
        out[b] = (res.results[2 * b]["out_partial"].astype(np.float32)
                  + res.results[2 * b + 1]["out_partial"].astype(np.float32)
                  + b_proj[None, :])
    return out


# revision 18
# speedup vs baseline: 1.0252x; 1.0252x over previous
"""Multi-head attention (B=4, N=2048, C=1024, H=16) on 8 TRN2 NeuronCores.

Sharding: core = 2*b + half handles batch b, heads half*8 .. half*8+7.
Each core computes QKV for its 8 heads, full attention for them, and a
partial projection (its 512 rows of W_proj). Host sums the two partials
per batch and adds the bias.

v3 schedule: the scalar engine's exp stream (256 x [128,1024]) is the
critical resource; everything else is arranged so neither it nor the
PE ever hits a head-of-line block:
  - x^T is staged in DRAM as 32 contiguous (query-block, c-chunk)
    blocks and DMA'd in dependency-chained groups, so K^T/Q^T for the
    first query block (and the exp stream) start after ~1 MB of
    traffic instead of after the full 4 MB.
  - softmax normalization is PE-free and off the critical path: the
    denominators ride the AV matmuls as a 65th stationary column; the
    sums row is evicted, spread across 128 partitions via a DRAM
    round-trip, reciprocal'd wide (~0.2us instead of 6.5us on one
    partition), broadcast back with a stride-0 DMA and multiplied in
    on DVE.  These DMAs ride the gpsimd (SWDGE) queue so the sync
    queue never blocks on them.
  - V (pairs 0-1 beyond the first tiles), V (pairs 2-3), Q/K for later
    pairs and the first 12 projection tiles are emitted as small fill
    units inside the attention blocks, sized to the PE slack there.
  - PSUM evictions go to the scalar engine only where it is idle
    (boot, projection tail), otherwise to DVE.

All matmul operands are fp16 (1 cycle/row on the PE), accumulation
fp32 in PSUM. Host pre-casts weights/x and pre-transposes x; output
partials return as fp16 and are summed on the host in fp32.
"""

import functools
from contextlib import ExitStack

import numpy as np

import concourse.bass as bass
import concourse.tile as tile
from concourse.tile import add_dep_helper
from concourse import bacc, mybir
from concourse.bass_utils import run_bass_kernel_spmd

F32 = mybir.dt.float32
F16 = mybir.dt.float16
AF = mybir.ActivationFunctionType

B, N, C = 4, 2048, 1024
H, D = 16, 64
P = 128
NCORES = 8
HPC = 8            # heads per core
PAIRS = HPC // 2   # 4
DCORE = HPC * D    # 512 attention columns per core
SCALE = float(H) ** -0.5  # 0.25 (faithful to reference: num_heads**-0.5)
EXP_BIAS = -5.0    # exp(scale*s + bias): cancels in softmax, keeps fp16 range
NB = N // 512      # 4 query blocks
NT = N // P        # 16 key tiles of 128
CT = C // P        # 8 contraction chunks
VW = D + 1         # V columns per head incl. the ones column (row sums)
MBLK = HPC * VW    # 520 v_sb columns per m-tile
HB = DCORE // 2    # 256 V columns per half (head pairs 0-1 / 2-3)

LAST_RESULT = None  # BassKernelResults of the most recent run (for test.py)


def _kernel_body(tc, out_d, xtb_d, wq_d, wk_d, wv_d, wp_d):
    nc = tc.nc
    with ExitStack() as ctx:
        const = ctx.enter_context(tc.tile_pool(name="const", bufs=1))
        ones_f = const.tile([P, P], F32)
        nc.vector.memset(ones_f, 1.0)
        ebias = const.tile([P, 1], F32)
        nc.vector.memset(ebias, EXP_BIAS)

        # attT: pair p occupies cols [p*N, (p+1)*N); partitions = 2 heads x 64
        attT_pool = ctx.enter_context(tc.tile_pool(name="attT", bufs=1))
        attT = attT_pool.tile([P, PAIRS * N], F16)
        xt_pool = ctx.enter_context(tc.tile_pool(name="xt", bufs=1))
        xt = xt_pool.tile([P, CT * N], F16)
        v_pool = ctx.enter_context(tc.tile_pool(name="v", bufs=1))
        v_sb = v_pool.tile([P, NT * MBLK], F16)
        wv_pool = ctx.enter_context(tc.tile_pool(name="wv", bufs=1))
        wv_sb = wv_pool.tile([P, CT * DCORE], F16)
        wp_pool = ctx.enter_context(tc.tile_pool(name="wp", bufs=1))
        wp_sb = wp_pool.tile([P, PAIRS * C], F16)

        qt_pool = ctx.enter_context(tc.tile_pool(name="qt", bufs=3))
        kt_pool = ctx.enter_context(tc.tile_pool(name="kt", bufs=3))
        wqk_pool = ctx.enter_context(tc.tile_pool(name="wqk", bufs=4))
        pt_pool = ctx.enter_context(tc.tile_pool(name="pt", bufs=8))
        tmb_pool = ctx.enter_context(tc.tile_pool(name="tmb", bufs=3))
        sums_pool = ctx.enter_context(tc.tile_pool(name="sums", bufs=2))
        spread_pool = ctx.enter_context(tc.tile_pool(name="spread", bufs=2))
        rb_pool = ctx.enter_context(tc.tile_pool(name="rb", bufs=2))
        stage_pool = ctx.enter_context(tc.tile_pool(name="stage", bufs=3))
        dram_pool = ctx.enter_context(
            tc.tile_pool(name="dscr", bufs=3, space="DRAM"))

        ps_mm = ctx.enter_context(tc.tile_pool(name="ps_mm", bufs=2, space="PSUM"))

        qt_tiles = [None] * PAIRS
        kt_tiles = [None] * PAIRS
        wt_tiles = [None] * PAIRS

        def emit_wqk_dma(p):
            # host pre-packs the SBUF image: block p is a contiguous
            # [128, CT*P] slab, so this is a single dense transfer
            tiles = []
            for w_d in (wq_d, wk_d):
                wt = wqk_pool.tile([P, CT * P], F16, tag="w")
                nc.sync.dma_start(out=wt, in_=w_d[p * P:(p + 1) * P, :])
                tiles.append(wt)
            wt_tiles[p] = tiles

        def emit_wv_dma(half):
            # half-major packed image: one contiguous [128, CT*HB] transfer
            w = CT * HB
            return [nc.sync.dma_start(
                out=wv_sb[:, half * w:(half + 1) * w],
                in_=wv_d[:, half * w:(half + 1) * w])]

        # ---- boot DMAs, dependency-chained so the first query block's
        # x^T (1 MB) and wv(pairs 0-1) land before the rest of x^T ----
        emit_wqk_dma(0)
        xt_last = {}

        def emit_xt_group(nb, after=None):
            # split each group across the HWDGE (sync) and SWDGE (gpsimd)
            # queue families to engage more DMA capacity during boot
            last = None
            for cc in range(CT):
                eng = nc.sync if cc % 2 == 0 else nc.gpsimd
                ins = eng.dma_start(
                    out=xt[:, cc * N + nb * 512: cc * N + nb * 512 + 512],
                    in_=xtb_d[(nb * CT + cc) * P:(nb * CT + cc + 1) * P, :])
                if after is not None:
                    add_dep_helper(ins.ins, after.ins, sync=True,
                                   reason="boot DMA pacing")
                last = ins
            xt_last[nb] = last
            return last

        g_a = emit_xt_group(0)
        wv01 = emit_wv_dma(0)
        for ins in wv01:
            add_dep_helper(ins.ins, g_a.ins, sync=True,
                           reason="boot DMA pacing")
        g_c = emit_xt_group(1, after=g_a)
        g_d = emit_xt_group(2, after=g_c)
        emit_xt_group(3, after=g_d)

        # ones columns of v_sb (fused softmax row sums)
        ones_cols = v_sb.rearrange("q (g k) -> q g k", k=VW)[:, :, D:VW]
        nc.vector.tensor_copy(
            ones_cols, ones_f.rearrange("q (g k) -> q g k", k=1))

        def emit_v_tile(half, m, evict_vector):
            base0 = half * CT * HB
            psv = ps_mm.tile([P, HB], F32, tag="mm")
            for cc in range(CT):
                nc.tensor.matmul(
                    psv,
                    xt[:, cc * N + m * P: cc * N + (m + 1) * P],
                    wv_sb[:, base0 + cc * HB: base0 + (cc + 1) * HB],
                    start=(cc == 0), stop=(cc == CT - 1))
            base = m * MBLK + 4 * half * VW
            dst = v_sb[:, base: base + 4 * VW].rearrange(
                "q (h k) -> q h k", k=VW)[:, :, 0:D]
            src = psv.rearrange("q (h k) -> q h k", k=D)
            if evict_vector:
                nc.vector.tensor_copy(dst, src)
            else:
                nc.scalar.copy(dst, src)

        def emit_qk_block(p, which, nb):
            # which: 0 = q, 1 = k
            wt = wt_tiles[p][which]
            dst = (qt_tiles if which == 0 else kt_tiles)[p]
            psq = ps_mm.tile([P, 512], F32, tag="mm")
            for cc in range(CT):
                nc.tensor.matmul(
                    psq,
                    wt[:, cc * P:(cc + 1) * P],
                    xt[:, cc * N + nb * 512: cc * N + nb * 512 + 512],
                    start=(cc == 0), stop=(cc == CT - 1))
            nc.vector.tensor_copy(dst[:, nb * 512:(nb + 1) * 512], psq)

        def emit_proj(i, evict_vector):
            for co in range(2):
                psp = ps_mm.tile([P, 512], F32, tag="mm")
                for dc in range(PAIRS):
                    nc.tensor.matmul(
                        psp,
                        attT[:, dc * N + i * P: dc * N + (i + 1) * P],
                        wp_sb[:, dc * C + co * 512: dc * C + co * 512 + 512],
                        start=(dc == 0), stop=(dc == PAIRS - 1))
                st = stage_pool.tile([P, 512], F16, tag="st")
                if evict_vector:
                    nc.vector.tensor_copy(st, psp)
                else:
                    nc.scalar.copy(st, psp)
                nc.sync.dma_start(
                    out=out_d[i * P:(i + 1) * P, co * 512: co * 512 + 512],
                    in_=st)

        # ---- boot PE work: K^T(p0,b0), Q^T(p0,b0), V(pairs 0-1, m 0-3) ----
        qt_tiles[0] = qt_pool.tile([P, N], F16, tag="qt", name="qt0")
        kt_tiles[0] = kt_pool.tile([P, N], F16, tag="kt", name="kt0")
        emit_qk_block(0, 1, 0)
        emit_qk_block(0, 0, 0)
        emit_v_tile(0, 0, evict_vector=True)
        emit_v_tile(0, 1, evict_vector=True)

        ps_s = ctx.enter_context(tc.tile_pool(name="ps_s", bufs=2, space="PSUM"))
        ps_av = ctx.enter_context(tc.tile_pool(name="ps_av", bufs=1, space="PSUM"))

        # ---- fill units: small chunks of PE work dropped into the slack
        # of the exp-bound attention blocks ----
        def qtf(p, nb):
            return lambda: emit_qk_block(p, 0, nb)

        def ktf(p, nb):
            return lambda: emit_qk_block(p, 1, nb)

        def vf01(m):
            return lambda: emit_v_tile(0, m, evict_vector=True)

        def vf23(m):
            return lambda: emit_v_tile(1, m, evict_vector=True)

        def projf(i):
            return lambda: emit_proj(i, evict_vector=True)

        def newpair(p):
            def f():
                qt_tiles[p] = qt_pool.tile([P, N], F16, tag="qt", name=f"qt{p}")
                kt_tiles[p] = kt_pool.tile([P, N], F16, tag="kt", name=f"kt{p}")
                emit_wqk_dma(p)
            return f

        def misc1():  # wv(pairs 2-3) + pair-1 weights
            def f():
                emit_wv_dma(1)
                newpair(1)()
            return f

        def misc2():  # pair-3 weights + W_proj
            def f():
                newpair(3)()
                for dc in range(PAIRS):
                    nc.sync.dma_start(out=wp_sb[:, dc * C:(dc + 1) * C],
                                      in_=wp_d[dc * P:(dc + 1) * P, :])
            return f

        FILLS = {
            (0, 0): [vf01(2), vf01(3), ktf(0, 1), vf01(4), vf01(5),
                     vf01(6), ktf(0, 2), vf01(7), vf01(8), ktf(0, 3),
                     vf01(9), vf01(10), qtf(0, 1), vf01(11), vf01(12),
                     vf01(13), vf01(14), vf01(15)],
            (0, 1): [misc1(), qtf(0, 2)],
            (0, 2): [qtf(0, 3), vf23(0), vf23(1), vf23(2)],
            (0, 3): [ktf(1, 0), ktf(1, 1), qtf(1, 0), vf23(3), vf23(4)],
            (1, 0): [ktf(1, 2), ktf(1, 3), qtf(1, 1), vf23(5), vf23(6)],
            (1, 1): [newpair(2), misc2(), qtf(1, 2), vf23(7), vf23(8)],
            (1, 2): [qtf(1, 3), ktf(2, 0), ktf(2, 1), vf23(9), vf23(10)],
            (1, 3): [ktf(2, 2), ktf(2, 3), qtf(2, 0), vf23(11), vf23(12)],
            (2, 0): [vf23(13), vf23(14), vf23(15), qtf(2, 1), ktf(3, 0)],
            (2, 1): [ktf(3, 1), ktf(3, 2), ktf(3, 3), qtf(3, 0), qtf(2, 2)],
            (3, 0): [qtf(2, 3), qtf(3, 1)],
            (2, 2): [qtf(3, 2), projf(0), projf(1)],
            (3, 1): [projf(2), projf(3), qtf(3, 3)],
            (3, 2): [projf(4), projf(5)],
            (2, 3): [projf(6), projf(7), projf(8)],
            (3, 3): [projf(9), projf(10), projf(11)],
        }
        # pair 2/3 interleave: pair-3 norms land early enough that the
        # projection spreads over the last five blocks instead of piling
        # up after the exp stream ends
        BLOCK_ORDER = ([(0, nb) for nb in range(NB)]
                       + [(1, nb) for nb in range(NB)]
                       + [(2, 0), (2, 1), (3, 0), (2, 2),
                          (3, 1), (3, 2), (2, 3), (3, 3)])
        # fills consumed per m-slot: (0,0) takes 2 per slot early (it is
        # DMA-paced anyway); other blocks 1 at each of 5 points.
        DENSE_FILL_CNT = {m: (2 if m < 8 else 1) for m in range(16)}
        FILL_CNT = {2: 1, 5: 1, 8: 1, 11: 1, 14: 1}

        def emit_c_block(p, nb):
            qt_t, kt_t = qt_tiles[p], kt_tiles[p]
            fills = list(FILLS.get((p, nb), []))
            fill_cnt = DENSE_FILL_CNT if (p, nb) == (0, 0) else FILL_CNT
            nsl = slice(nb * 512, nb * 512 + 512)
            osl = slice(p * N + nb * 512, p * N + nb * 512 + 512)
            ps_av_t = ps_av.tile([P, 1024], F32, tag="av")
            def emit_av(m, pt):
                first, last = (m == 0), (m == NT - 1)
                vbase = m * MBLK + 2 * p * VW
                nc.tensor.matmul(
                    ps_av_t[0:VW, 0:512],
                    v_sb[:, vbase: vbase + VW],
                    pt[:, 0:512],
                    start=first, stop=last, skip_group_check=True)
                nc.tensor.matmul(
                    ps_av_t[0:VW, 512:1024],
                    v_sb[:, vbase + VW: vbase + 2 * VW],
                    pt[:, 512:1024],
                    start=first, stop=last, skip_group_check=True)

            # AV trails scores/exp by 2 m-steps so the AV-start WAR on the
            # previous block's eviction never blocks the score stream
            av_q = []
            for m in range(NT):
                for _ in range(fill_cnt.get(m, 0)):
                    if fills:
                        fills.pop(0)()
                ps_s_t = ps_s.tile([P, 1024], F32, tag="s")
                nc.tensor.matmul(
                    ps_s_t[:, 0:512],
                    kt_t[0:64, m * P:(m + 1) * P], qt_t[0:64, nsl],
                    start=True, stop=True)
                nc.tensor.matmul(
                    ps_s_t[:, 512:1024],
                    kt_t[64:128, m * P:(m + 1) * P], qt_t[64:128, nsl],
                    start=True, stop=True)
                pt = pt_pool.tile([P, 1024], F16, tag="pt")
                nc.scalar.activation(pt, ps_s_t, AF.Exp,
                                     scale=SCALE, bias=ebias)
                av_q.append((m, pt))
                if len(av_q) > 2:
                    emit_av(*av_q.pop(0))
            while fills:
                fills.pop(0)()
            while av_q:
                emit_av(*av_q.pop(0))
            # evict each PSUM bank with ONE fp16 copy (rows 0-63 head data,
            # row 64 the softmax sums), so the next block's AV start
            # unblocks after ~0.7us per bank
            dmae = nc.sync if p == PAIRS - 1 else nc.gpsimd
            stA = sums_pool.tile([P, 512], F16, tag="stA")
            nc.vector.tensor_copy(stA[0:VW, :], ps_av_t[0:VW, 0:512])
            stB = sums_pool.tile([P, 512], F16, tag="stB")
            nc.vector.tensor_copy(stB[0:VW, :], ps_av_t[0:VW, 512:1024])
            dsum = dram_pool.tile([1024], F16, tag="dsum")
            dmae.dma_start(out=dsum.rearrange("(a b) -> a b", a=1)[:, 0:512],
                           in_=stA[64:65, :])
            dmae.dma_start(out=dsum.rearrange("(a b) -> a b", a=1)[:, 512:1024],
                           in_=stB[64:65, :])
            # normalization (PE-free): spread sums across 128 partitions via
            # DRAM, wide reciprocal, stride-0 broadcast back, multiply in.
            spread = spread_pool.tile([P, 8], F16, tag="spf")
            dmae.dma_start(out=spread,
                           in_=dsum.rearrange("(q f) -> q f", q=P))
            spreadr = spread_pool.tile([P, 8], F16, tag="sph")
            with nc.allow_low_precision(reason="softmax recip rounding"):
                nc.vector.reciprocal(spreadr, spread)
            drec = dram_pool.tile([1024], F16, tag="drec")
            dmae.dma_start(out=drec.rearrange("(q f) -> q f", q=P),
                           in_=spreadr)
            rb = rb_pool.tile([64, 1024], F16, tag="rb")
            dmae.dma_start(
                out=rb,
                in_=drec.rearrange("(a b) -> a b", a=1).broadcast_to([64, 1024]))
            nc.vector.tensor_mul(attT[0:64, osl], stA[0:64, :], rb[:, 0:512])
            tmb = tmb_pool.tile([64, 512], F16, tag="tmb")
            nc.vector.tensor_mul(tmb, stB[0:64, :], rb[:, 512:1024])
            nc.sync.dma_start(out=attT[64:128, osl], in_=tmb)

        for p, nb in BLOCK_ORDER:
            emit_c_block(p, nb)

        # ---- tail: projection of the last query block ----
        for i in range(4 * (NB - 1), 4 * NB):
            emit_proj(i, evict_vector=False)


@functools.lru_cache(maxsize=1)
def build_nc():
    nc = bacc.Bacc("TRN2", target_bir_lowering=False, debug=False)
    xtb_d = nc.dram_tensor("xt_blocks", [NB * CT * P, 512], F16,
                           kind="ExternalInput").ap()
    wq_d = nc.dram_tensor("wq", [PAIRS * P, CT * P], F16,
                          kind="ExternalInput").ap()
    wk_d = nc.dram_tensor("wk", [PAIRS * P, CT * P], F16,
                          kind="ExternalInput").ap()
    wv_d = nc.dram_tensor("wv", [P, 2 * CT * HB], F16,
                          kind="ExternalInput").ap()
    wp_d = nc.dram_tensor("wp", [DCORE, C], F16, kind="ExternalInput").ap()
    out_d = nc.dram_tensor("out_partial", [N, C], F16, kind="ExternalOutput").ap()
    with tile.TileContext(nc) as tc:
        _kernel_body(tc, out_d, xtb_d, wq_d, wk_d, wv_d, wp_d)
    nc.compile()
    return nc


def make_in_maps(x, W_qkv, W_proj):
    in_maps = []
    for core in range(NCORES):
        b, half = core // 2, core % 2
        h0 = half * HPC
        xt = x[b].T.astype(np.float16)              # [C, N]
        xtb = np.ascontiguousarray(
            xt.reshape(CT, P, NB, 512).transpose(2, 0, 1, 3)
        ).reshape(NB * CT * P, 512)

        def pack_qk(w):                              # [C, DCORE] -> SBUF image
            blocks = []
            for p_ in range(PAIRS):
                blk = w[:, p_ * P:(p_ + 1) * P]      # [C, 128]
                blocks.append(blk.reshape(CT, P, P).transpose(1, 0, 2)
                              .reshape(P, CT * P))
            return np.ascontiguousarray(np.concatenate(blocks, axis=0))

        wq_full = W_qkv[:, 0 * C + h0 * D: 0 * C + h0 * D + DCORE].astype(np.float16)
        wk_full = W_qkv[:, 1 * C + h0 * D: 1 * C + h0 * D + DCORE].astype(np.float16)
        wv_full = W_qkv[:, 2 * C + h0 * D: 2 * C + h0 * D + DCORE].astype(np.float16)
        wv_img = np.ascontiguousarray(
            wv_full.reshape(CT, P, 2, HB).transpose(1, 2, 0, 3)
        ).reshape(P, 2 * CT * HB)
        in_maps.append({
            "xt_blocks": xtb,
            "wq": pack_qk(wq_full),
            "wk": pack_qk(wk_full),
            "wv": wv_img,
            "wp": np.ascontiguousarray(
                W_proj[h0 * D: h0 * D + DCORE, :].astype(np.float16)),
        })
    return in_maps


def kernel(x, W_qkv, W_proj, b_proj, trace=False):
    x = np.asarray(x, dtype=np.float32)
    W_qkv = np.asarray(W_qkv, dtype=np.float32)
    W_proj = np.asarray(W_proj, dtype=np.float32)
    b_proj = np.asarray(b_proj, dtype=np.float32)

    nc = build_nc()
    in_maps = make_in_maps(x, W_qkv, W_proj)

    global LAST_RESULT
    res = run_bass_kernel_spmd(nc, in_maps, list(range(NCORES)), trace=trace)
    LAST_RESULT = res

    out = np.empty((B, N, C), dtype=np.float32)
    for b in range(B):
        out[b] = (res.results[2 * b]["out_partial"].astype(np.float32)
                  + res.results[2 * b + 1]["out_partial"].astype(np.float32)
                  + b_proj[None, :])
    return out
